# revision 21
# baseline (speedup 1.0000x reference)
# GAT (2-layer, PyG-faithful) on 8 Trainium2 NeuronCores.
#
# Strategy (graph/data parallel, per sharding hint):
#  - Nodes padded to NPAD = 8*NSH; core k owns dst nodes [k*NSH, (k+1)*NSH).
#  - Edges partitioned by dst core, chunked into 128-edge chunks per 128-dst
#    tile (lo/hi split at src=32768 for int16 gather indices).
#  - Per layer: per-shard rows [h bf16 | a_src bf16] are written to a
#    256B-strided table, AllGathered, then per-edge rows fetched with
#    dma_gather reading only the used prefix (520B/336B) of each row.
#  - Segment softmax denominators and weighted message sums accumulate in
#    PSUM via one-hot matmuls. One-hot matrices (OH: edge->dst scatter;
#    OHT: per-edge a_dst select) are precomputed on host in fp8 and
#    streamed from DRAM (graph is static).
#  - Feature standardization is folded into W1 (rows scaled by 1/std, bias
#    via K=1 matmul of -mean@W1), so x is consumed only as xT (host-
#    pretransposed) by bf16 stage-1 matmuls. Column stats come from ACT
#    passes with accum_out + a tiny AllReduce.
#  - Self-loops are handled analytically per dst tile (no gather needed).
import math
from dataclasses import dataclass, field

import numpy as np
import ml_dtypes

import concourse.bass as bass
import concourse.bacc as bacc
import concourse.tile as tile
from concourse import mybir
from concourse import bass_utils
from concourse import ap_utils
from concourse.bass import MemorySpace
from concourse.masks import make_identity

F32 = mybir.dt.float32
BF16 = mybir.dt.bfloat16
FP8 = mybir.dt.float8e4
I16 = mybir.dt.int16
AOP = mybir.AluOpType
ACT = mybir.ActivationFunctionType
NEG = 0.2
NP_FP8 = ml_dtypes.float8_e4m3


def dma_gather_raw(eng, out_ap, in_ap, idxs_ap, num_idxs, elem_size, elem_step,
                   queue_num=0):
    """dma_gather with elem_size free of the %256 restriction (which only
    applies to transpose mode); elem_step (table row stride) must be %256B."""
    assert idxs_ap.dtype == mybir.dt.int16
    assert in_ap.dtype == out_ap.dtype
    assert in_ap.space == MemorySpace.DRAM
    assert idxs_ap.space == MemorySpace.SBUF
    assert out_ap.space == MemorySpace.SBUF
    assert ap_utils.ap_is_contiguous(in_ap.ap[1:])
    assert ap_utils.ap_is_contiguous(out_ap.ap[1:])
    assert ap_utils.ap_is_contiguous(idxs_ap.ap[1:])
    assert in_ap.ap[-1][1] == elem_size == out_ap.ap[-1][1]
    assert in_ap.ap[0][0] == elem_step
    stride_bytes = elem_step * mybir.dt.size(in_ap.dtype)
    stride_bytes_256 = stride_bytes // 256
    assert stride_bytes % 256 == 0 and stride_bytes_256 < 256
    return eng.add_instruction(
        mybir.InstDMAGatherAnt(
            name=eng.bass.get_next_instruction_name(),
            ins=[*eng.lower_ap_dma(in_ap, for_custom_bir_dma=True),
                 eng.lower_ap(idxs_ap),
                 eng.lower_val_access(eng.to_reg(num_idxs))],
            outs=[eng.lower_ap(out_ap)],
            transpose=False,
            num_idxs=num_idxs,
            elem_size=elem_size,
            stride_bytes_256=stride_bytes_256,
            gen_mode=0,
            single_packet=False,
            queue_num=queue_num,
        ))


@dataclass
class Cfg:
    N: int = 50000
    FIN: int = 128
    H: int = 4
    HID: int = 64          # layer-1 per-head dim
    CLS: int = 40          # layer-2 per-head dim
    NCORES: int = 8
    SPLIT: int = 24576  # = NCORES * 24*128 (half-A rows)
    GROUP: int = 2         # dst tiles per gather-call group

    @property
    def F1(self):
        return self.H * self.HID

    @property
    def F2(self):
        return self.H * self.CLS

    @property
    def NSH(self):
        return math.ceil(self.N / (self.NCORES * 128)) * 128

    @property
    def NPAD(self):
        return self.NSH * self.NCORES

    @property
    def T(self):
        return self.NSH // 128

    @property
    def ROW1S(self):  # bf16 elems per hext1 table row (stride, %128 for 256B)
        return math.ceil((self.F1 + self.H) / 128) * 128

    @property
    def ROW1E(self):  # bf16 elems gathered per row
        return self.F1 + self.H

    @property
    def ROW2S(self):
        return math.ceil((self.F2 + self.H) / 128) * 128

    @property
    def ROW2E(self):
        return self.F2 + self.H


@dataclass
class GroupMeta:
    tiles: list
    lo_off: int          # column offset into idxlo array (int16 cols)
    nlo: int
    hi_off: int
    nhi: int
    ad_off: int          # chunk-slot offset (one-hot tables / gather chunks)
    nad: int
    tile_chunks: list = field(default_factory=list)


def build_plan(cfg: Cfg, edge_index: np.ndarray):
    """Partition edges; equalize chunk counts across cores (SPMD program is
    shared). Returns (groups_meta, per-core input dict, sizes)."""
    src = edge_index[0].astype(np.int64)
    dst = edge_index[1].astype(np.int64)
    NSH, T, NC = cfg.NSH, cfg.T, cfg.NCORES

    # half-major remap: halves (tiles [0,24) / [24,T)) of each shard are
    # grouped so each AllGather half is contiguous in the full table
    HA = 24 * 128
    ids = np.arange(cfg.NPAD, dtype=np.int64)
    kk, rr = ids // NSH, ids % NSH
    remap = np.where(rr < HA, kk * HA + rr,
                     NC * HA + kk * (NSH - HA) + (rr - HA))
    src = remap[src]

    core = dst // NSH
    tloc = (dst % NSH) // 128
    is_lo = src < cfg.SPLIT

    lists = [[[None, None] for _ in range(T)] for _ in range(NC)]
    order = np.lexsort((src, tloc, core))
    so_src, so_dst, so_core, so_tloc, so_lo = (
        src[order], dst[order], core[order], tloc[order], is_lo[order])
    for k in range(NC):
        mk = so_core == k
        for t in range(T):
            mt = mk & (so_tloc == t)
            ml = mt & so_lo
            mh = mt & ~so_lo
            lists[k][t][0] = (so_src[ml], so_dst[ml])
            lists[k][t][1] = (so_src[mh], so_dst[mh])

    Cl = [max(math.ceil(len(lists[k][t][0][0]) / 128) for k in range(NC)) for t in range(T)]
    Ch = [max(math.ceil(len(lists[k][t][1][0]) / 128) for k in range(NC)) for t in range(T)]
    Cl = [max(c, 1) for c in Cl]
    Ch = [max(c, 1) for c in Ch]

    G = cfg.GROUP
    groups = []
    lo_off = hi_off = ad_off = 0
    for g0 in range(0, T, G):
        tiles = list(range(g0, min(g0 + G, T)))
        nlo = sum(Cl[t] for t in tiles)
        nhi = sum(Ch[t] for t in tiles)
        nad = nlo + nhi
        gm = GroupMeta(tiles, lo_off, nlo, hi_off, nhi, ad_off, nad)
        slot = 0
        lo_slots = {}
        for t in tiles:
            lo_slots[t] = list(range(slot, slot + Cl[t]))
            slot += Cl[t]
        hi_slots = {}
        for t in tiles:
            hi_slots[t] = list(range(slot, slot + Ch[t]))
            slot += Ch[t]
        for t in tiles:
            gm.tile_chunks.append((t, lo_slots[t] + hi_slots[t]))
        groups.append(gm)
        lo_off += nlo * 8
        hi_off += nhi * 8
        ad_off += nad

    SLO, SHI, NCH = lo_off, hi_off, ad_off

    def wrap16(vals):
        n = len(vals)
        a = np.zeros((16, n // 16), np.int16)
        a[np.arange(n) % 16, np.arange(n) // 16] = vals
        return np.tile(a, (8, 1))

    per_core = []
    for k in range(NC):
        idxlo = np.zeros((128, SLO), np.int16)
        idxhi = np.zeros((128, SHI), np.int16)
        dl_all = np.full((128, NCH), -1.0, np.float32)
        for gm in groups:
            lo_stream = []
            hi_stream = []
            dl = np.full((128, gm.nad), -1.0, np.float32)
            slot = 0
            for cls in (0, 1):
                for t in gm.tiles:
                    s_, d_ = lists[k][t][cls]
                    nch = Cl[t] if cls == 0 else Ch[t]
                    npadded = nch * 128
                    sp = np.zeros(npadded, np.int64)
                    sp[:len(s_)] = s_ if cls == 0 else s_ - cfg.SPLIT
                    dlp = np.full(npadded, -1.0, np.float32)
                    dlp[:len(d_)] = (d_ % NSH) % 128
                    (lo_stream if cls == 0 else hi_stream).append(sp)
                    dl[:, slot:slot + nch] = dlp.reshape(nch, 128).T
                    slot += nch
            lo_v = np.concatenate(lo_stream) if lo_stream else np.zeros(0, np.int64)
            hi_v = np.concatenate(hi_stream) if hi_stream else np.zeros(0, np.int64)
            if len(lo_v):
                idxlo[:, gm.lo_off:gm.lo_off + gm.nlo * 8] = wrap16(lo_v)
            if len(hi_v):
                idxhi[:, gm.hi_off:gm.hi_off + gm.nhi * 8] = wrap16(hi_v)
            dl_all[:, gm.ad_off:gm.ad_off + gm.nad] = dl
        # one-hot tables, fp8, interleaved per slot: [oh | oht].
        #   oh[p, s, d] = (dl[p, s] == d)   (edge->dst scatter)
        #   oht[p, s, e] = (dl[e, s] == p)  (dst-select per edge)
        ar = np.arange(128, dtype=np.float32)
        ohc = np.zeros((128, NCH, 2, 128), NP_FP8)
        ohc[:, :, 0, :] = (dl_all[:, :, None] == ar[None, None, :]).astype(NP_FP8)
        ohc[:, :, 1, :] = (ar[:, None, None] == dl_all.T[None, :, :]).astype(NP_FP8)
        per_core.append(dict(idxlo=idxlo, idxhi=idxhi,
                             ohc=np.ascontiguousarray(ohc.reshape(128, NCH * 256))))
    return groups, per_core, (SLO, SHI, NCH)


def build_program(cfg: Cfg, groups, sizes):
    SLO, SHI, NCH = sizes
    H, F1, F2, HID, CLS = cfg.H, cfg.F1, cfg.F2, cfg.HID, cfg.CLS
    NSH, NPAD, T = cfg.NSH, cfg.NPAD, cfg.T
    ROW1S, ROW1E, ROW2S, ROW2E = cfg.ROW1S, cfg.ROW1E, cfg.ROW2S, cfg.ROW2E
    K1 = cfg.FIN
    assert K1 == 128
    NREAL = cfg.N

    nc = bacc.Bacc("TRN2", target_bir_lowering=False, debug=False,
                   num_devices=cfg.NCORES, num_swdge_queues=4)
    t_xT = nc.dram_tensor("xT", [128, NSH], F32, kind="ExternalInput").ap()
    t_W1 = nc.dram_tensor("W1", [K1, F1], F32, kind="ExternalInput").ap()
    t_A1 = nc.dram_tensor("A1", [128, 2 * 2 * H], F32, kind="ExternalInput").ap()
    t_b1 = nc.dram_tensor("b1", [1, F1], F32, kind="ExternalInput").ap()
    t_W2 = nc.dram_tensor("W2", [128, 2 * F2], F32, kind="ExternalInput").ap()
    t_A2 = nc.dram_tensor("A2", [128, 2 * 2 * H], F32, kind="ExternalInput").ap()
    t_b2 = nc.dram_tensor("b2", [1, F2], F32, kind="ExternalInput").ap()
    t_idxlo = nc.dram_tensor("idxlo", [128, SLO], I16, kind="ExternalInput").ap()
    t_idxhi = nc.dram_tensor("idxhi", [128, SHI], I16, kind="ExternalInput").ap()
    t_ohc = nc.dram_tensor("ohc", [128, NCH * 256], FP8, kind="ExternalInput").ap()
    t_out = nc.dram_tensor("out", [NSH, F2], F32, kind="ExternalOutput").ap()

    with tile.TileContext(nc) as tc:
        const = tc.alloc_tile_pool(name="const", bufs=1)
        dram = tc.alloc_tile_pool(name="dram", bufs=1, space="DRAM")

        HA = 24 * 128
        HB = NSH - HA
        hext1_localA = dram.tile([HA, ROW1S], BF16)
        hext1_localB = dram.tile([HB, ROW1S], BF16)
        hext1_fullA = dram.tile([cfg.NCORES * HA, ROW1S], BF16, addr_space="Shared")
        hext1_fullB = dram.tile([cfg.NCORES * HB, ROW1S], BF16, addr_space="Shared")
        hext2_localA = dram.tile([HA, ROW2S], BF16)
        hext2_localB = dram.tile([HB, ROW2S], BF16)
        hext2_fullA = dram.tile([cfg.NCORES * HA, ROW2S], BF16, addr_space="Shared")
        hext2_fullB = dram.tile([cfg.NCORES * HB, ROW2S], BF16, addr_space="Shared")

        def hloc_rows(hA, hB, t):
            if t < 24:
                return hA, 128 * t
            return hB, 128 * (t - 24)

        ident = const.tile([128, 128], F32)
        make_identity(nc, ident)
        b1b = const.tile([128, F1], F32)
        nc.gpsimd.dma_start(out=b1b[:], in_=t_b1.to_broadcast([128, F1]))
        b2b = const.tile([128, F2], F32)
        nc.gpsimd.dma_start(out=b2b[:], in_=t_b2.to_broadcast([128, F2]))
        ones1 = const.tile([1, 128], BF16)
        nc.vector.memset(ones1[:], 1.0)

        # ---- W1ext [128, F1 + 2H] = [W1 | W1 @ A1blocks] ----
        with tc.tile_pool(name="wtmp", bufs=1) as wtmp, \
             tc.tile_pool(name="wpsum", bufs=1, space="PSUM") as wpsum:
            W1sb = const.tile([128, F1], F32)
            nc.sync.dma_start(out=W1sb[:], in_=t_W1[:])
            A1sb = wtmp.tile([128, 2 * 2 * H], F32, tag="a")
            nc.sync.dma_start(out=A1sb[:], in_=t_A1[:])
            n1b = F1 // 128
            W1A_ps = wpsum.tile([128, 2 * H], F32, tag="wa")
            for b in range(n1b):
                trp = wpsum.tile([128, 128], F32, tag="tr")
                nc.tensor.transpose(out=trp[:], in_=W1sb[:, 128 * b:128 * (b + 1)],
                                    identity=ident[:])
                trs = wtmp.tile([128, 128], F32, tag="trs")
                nc.vector.tensor_copy(out=trs[:], in_=trp[:])
                nc.tensor.matmul(W1A_ps[:], lhsT=trs[:],
                                 rhs=A1sb[:, 2 * H * b:2 * H * (b + 1)],
                                 start=(b == 0), stop=(b == n1b - 1))
            W1ext = const.tile([128, F1 + 2 * H], F32)
            nc.vector.tensor_copy(out=W1ext[:, 0:F1], in_=W1sb[:])
            nc.vector.tensor_copy(out=W1ext[:, F1:F1 + 2 * H], in_=W1A_ps[:])

            # ---- W2ext [128, 2, F2 + 2H] ----
            W2sb = const.tile([128, 2, F2], F32)
            nc.sync.dma_start(out=W2sb[:], in_=t_W2.rearrange("k (b f) -> k b f", b=2))
            A2sb = wtmp.tile([128, 2 * 2 * H], F32, tag="a")
            nc.sync.dma_start(out=A2sb[:], in_=t_A2[:])
            W2ext = const.tile([128, 2, F2 + 2 * H], F32)
            fo_blocks = [(0, 128)] + ([(128, F2 - 128)] if F2 > 128 else [])
            for fb in range(2):
                W2A_ps = wpsum.tile([128, 2 * H], F32, tag="wa")
                for bi, (fo0, fow) in enumerate(fo_blocks):
                    trp = wpsum.tile([128, 128], F32, tag="tr")
                    nc.tensor.transpose(out=trp[:fow, :],
                                        in_=W2sb[:, fb, fo0:fo0 + fow],
                                        identity=ident[:])
                    trs = wtmp.tile([128, 128], F32, tag="trs")
                    nc.vector.tensor_copy(out=trs[:fow, :], in_=trp[:fow, :])
                    nc.tensor.matmul(W2A_ps[:], lhsT=trs[:fow, :],
                                     rhs=A2sb[0:fow, 2 * H * bi:2 * H * (bi + 1)],
                                     start=(bi == 0), stop=(bi == len(fo_blocks) - 1))
                nc.vector.tensor_copy(out=W2ext[:, fb, 0:F2], in_=W2sb[:, fb, :])
                nc.vector.tensor_copy(out=W2ext[:, fb, F2:F2 + 2 * H], in_=W2A_ps[:])
            W2extb = const.tile([128, 2, F2 + 2 * H], BF16)
            nc.vector.tensor_copy(out=W2extb[:], in_=W2ext[:])

        # ---- column stats from xT; fold standardization into W1ext ----
        xp = tc.alloc_tile_pool(name="xp", bufs=1)
        xT = xp.tile([128, NSH], F32)
        for q, (a, b) in enumerate([(0, NSH // 2), (NSH // 2, NSH)]):
            eng = nc.sync if q == 0 else nc.scalar
            eng.dma_start(out=xT[:, a:b], in_=t_xT[:, a:b])
        W1extb = const.tile([128, F1 + 2 * H], BF16)
        bshb = const.tile([1, F1 + 2 * H], BF16)
        with tc.tile_pool(name="stt", bufs=1) as stt, \
             tc.tile_pool(name="stp", bufs=1, space="PSUM") as stp:
            sq = xp.tile([128, NSH], F32, tag="sq")
            s12 = stt.tile([128, 2], F32, tag="s12")
            nc.scalar.activation(out=sq[:], in_=xT[:], func=ACT.Copy,
                                 accum_out=s12[:, 0:1])
            nc.scalar.activation(out=sq[:], in_=xT[:], func=ACT.Square,
                                 accum_out=s12[:, 1:2])
            stat_in = dram.tile([128, 2], F32)
            stat_out = dram.tile([128, 2], F32, addr_space="Shared")
            nc.gpsimd.dma_start(out=stat_in[:], in_=s12[:])
            nc.gpsimd.collective_compute(
                "AllReduce", AOP.add, replica_groups=[list(range(cfg.NCORES))],
                ins=[stat_in.opt()], outs=[stat_out.opt()])
            sall = stt.tile([128, 2], F32, tag="sall")
            nc.sync.dma_start(out=sall[:], in_=stat_out[:])
            mean = stt.tile([128, 1], F32, tag="mean")
            nc.scalar.mul(mean[:], sall[:, 0:1], 1.0 / NREAL)
            ex2 = stt.tile([128, 1], F32, tag="ex2")
            nc.scalar.mul(ex2[:], sall[:, 1:2], 1.0 / NREAL)
            m2 = stt.tile([128, 1], F32, tag="m2")
            nc.vector.tensor_mul(out=m2[:], in0=mean[:], in1=mean[:])
            var = stt.tile([128, 1], F32, tag="var")
            nc.vector.tensor_tensor(out=var[:], in0=ex2[:], in1=m2[:], op=AOP.subtract)
            nc.scalar.mul(var[:], var[:], NREAL / (NREAL - 1.0))
            std = stt.tile([128, 1], F32, tag="std")
            nc.scalar.activation(out=std[:], in_=var[:], func=ACT.Sqrt)
            rstd = stt.tile([128, 1], F32, tag="rstd")
            nc.vector.reciprocal(out=rstd[:], in_=std[:])
            # scale W1ext rows by 1/std (applies to both W and W@A blocks)
            nc.vector.tensor_scalar(W1extb[:], W1ext[:], rstd[:], None, AOP.mult)
            negm = stt.tile([128, 1], BF16, tag="negm")
            nc.scalar.mul(negm[:], mean[:], -1.0)
            bsh_ps = stp.tile([1, F1 + 2 * H], F32, tag="bsh")
            nc.tensor.matmul(bsh_ps[:], lhsT=negm[:], rhs=W1extb[:],
                             start=True, stop=True)
            nc.vector.tensor_copy(out=bshb[:], in_=bsh_ps[:])

        xTb = xp.tile([128, NSH], BF16, tag="xtb")
        nc.vector.tensor_copy(out=xTb[:], in_=xT[:])

        ad1res = const.tile([128, T, H], BF16)
        ad2res = const.tile([128, T, H], BF16)

        idxlo_sb = const.tile([128, SLO], I16)
        nc.sync.dma_start(out=idxlo_sb[:], in_=t_idxlo[:])
        idxhi_sb = const.tile([128, SHI], I16)
        nc.sync.dma_start(out=idxhi_sb[:], in_=t_idxhi[:])



        # ---- stage 1: hext1 rows (h1 = xT.T @ W1ext + bsh) ----
        with tc.tile_pool(name="s1sb", bufs=3) as s1sb, \
             tc.tile_pool(name="s1ps", bufs=3, space="PSUM") as s1ps:
            for g0 in range(0, T, 2):
                ts = list(range(g0, min(g0 + 2, T)))
                hx = s1sb.tile([128, len(ts), ROW1S], BF16, tag="hx")
                nc.vector.memset(hx[:, :, ROW1E:ROW1S], 0)
                for i, t in enumerate(ts):
                    h1p = s1ps.tile([128, F1 + 2 * H], F32, tag="h1")
                    nc.tensor.matmul(h1p[:], lhsT=xTb[:, 128 * t:128 * (t + 1)],
                                     rhs=W1extb[:], start=True, stop=False)
                    nc.tensor.matmul(h1p[:], lhsT=ones1[:], rhs=bshb[:],
                                     start=False, stop=True)
                    nc.scalar.copy(hx[:, i, 0:F1], h1p[:, 0:F1])
                    nc.scalar.copy(hx[:, i, F1:F1 + H], h1p[:, F1:F1 + H])
                    nc.scalar.copy(ad1res[:, t, :], h1p[:, F1 + H:F1 + 2 * H])
                htgt, roff = hloc_rows(hext1_localA, hext1_localB, g0)
                nc.sync.dma_start(
                    out=htgt[roff:roff + 128 * len(ts), :].rearrange(
                        "(a p) r -> p a r", p=128),
                    in_=hx[:])
        xp.release()

        nc.gpsimd.collective_compute(
            "AllGather", AOP.bypass, replica_groups=[list(range(cfg.NCORES))],
            ins=[hext1_localA.opt()], outs=[hext1_fullA.opt()])
        nc.gpsimd.collective_compute(
            "AllGather", AOP.bypass, replica_groups=[list(range(cfg.NCORES))],
            ins=[hext1_localB.opt()], outs=[hext1_fullB.opt()])

        qrr = [0]

        def agg_layer(layer):
            if layer == 1:
                ROWS, ROWE, F, C, hfA, hfB, hlA, hlB, adres = (
                    ROW1S, ROW1E, F1, HID, hext1_fullA, hext1_fullB,
                    hext1_localA, hext1_localB, ad1res)
            else:
                ROWS, ROWE, F, C, hfA, hfB, hlA, hlB, adres = (
                    ROW2S, ROW2E, F2, CLS, hext2_fullA, hext2_fullB,
                    hext2_localA, hext2_localB, ad2res)
            with tc.tile_pool(name=f"ag{layer}", bufs=3) as ag, \
                 tc.tile_pool(name=f"agp{layer}", bufs=2, space="PSUM") as agp, \
                 tc.tile_pool(name=f"ep{layer}", bufs=3) as ep, \
                 tc.tile_pool(name=f"epp{layer}", bufs=2, space="PSUM") as epp:
                for gm in groups:
                    nlo, nhi, nad = gm.nlo, gm.nhi, gm.nad
                    il = idxlo_sb[:, gm.lo_off:gm.lo_off + nlo * 8]
                    ih = idxhi_sb[:, gm.hi_off:gm.hi_off + nhi * 8]
                    OHC = ag.tile([128, nad, 2, 128], FP8, tag="ohc")
                    nc.sync.dma_start(
                        out=OHC[:].rearrange("p n t e -> p (n t e)"),
                        in_=t_ohc[:, gm.ad_off * 256:(gm.ad_off + nad) * 256])
                    P = ag.tile([128, nad, ROWE], BF16, tag="p")
                    in_lo = bass.AP(tensor=hfA.tensor, offset=hfA[:].offset,
                                    ap=[[ROWS, cfg.NCORES * HA], [1, ROWE]])
                    in_hi = bass.AP(tensor=hfB.tensor, offset=hfB[:].offset,
                                    ap=[[ROWS, cfg.NCORES * HB], [1, ROWE]])
                    dma_gather_raw(nc.gpsimd, P[:, 0:nlo, :], in_lo, il,
                                   nlo * 128, ROWE, ROWS, queue_num=qrr[0] % 4)
                    qrr[0] += 1
                    dma_gather_raw(nc.gpsimd, P[:, nlo:nad, :], in_hi, ih,
                                   nhi * 128, ROWE, ROWS, queue_num=qrr[0] % 4)
                    qrr[0] += 1

                    # per-edge a_dst via PE: oht.T @ adst_tile
                    slot_tile = {}
                    for (t_, slots_) in gm.tile_chunks:
                        for s_ in slots_:
                            slot_tile[s_] = t_
                    adx = agp.tile([128, nad * H], F32, tag="adx")
                    for s in range(nad):
                        nc.tensor.matmul(adx[:, H * s:H * (s + 1)],
                                         lhsT=OHC[:, s, 1, :],
                                         rhs=adres[:, slot_tile[s], :],
                                         start=True, stop=True)

                    E1 = ag.tile([128, nad, H], F32, tag="e1")
                    adxv = adx[:].rearrange("p (n h) -> p n h", h=H)
                    nc.vector.tensor_tensor(
                        out=E1[:], in0=P[:, :, F:F + H], in1=adxv, op=AOP.add)
                    nc.vector.scalar_tensor_tensor(
                        out=E1[:], in0=E1[:], scalar=NEG, in1=E1[:],
                        op0=AOP.mult, op1=AOP.max)

                    # R payload = exp(E1) broadcast over C (ACT), then *= P (DVE 2x)
                    R = ag.tile([128, nad, F + H], BF16, tag="r")
                    e1b = bass.AP(tensor=E1.tensor, offset=E1[:].offset,
                                  ap=[E1[:].ap[0], [H, nad], [1, H], [0, C]])
                    nc.scalar.activation(
                        out=R[:, :, 0:F].rearrange("p n (h c) -> p n h c", h=H),
                        in_=e1b, func=ACT.Exp)
                    nc.scalar.activation(out=R[:, :, F:F + H], in_=E1[:],
                                         func=ACT.Exp)
                    nc.vector.tensor_tensor(
                        out=R[:, :, 0:F], in0=R[:, :, 0:F],
                        in1=P[:, :, 0:F], op=AOP.mult)

                    for ti, (t, slots) in enumerate(gm.tile_chunks):
                        ps = agp.tile([128, F + H], F32, tag="acc")
                        for si, s in enumerate(slots):
                            nc.tensor.matmul(ps[:], lhsT=OHC[:, s, 0, :], rhs=R[:, s, :],
                                             start=(si == 0), stop=(si == len(slots) - 1))
                        # ---- epilogue for tile t ----
                        hog = ep.tile([128, ROWE], BF16, tag="hog")
                        hsrc, hroff = hloc_rows(hlA, hlB, t)
                        nc.scalar.dma_start(out=hog[:],
                                            in_=hsrc[hroff:hroff + 128, 0:ROWE])
                        es = ep.tile([128, H], F32, tag="es")
                        nc.vector.tensor_tensor(
                            out=es[:], in0=hog[:, F:F + H],
                            in1=adres[:, t, :], op=AOP.add)
                        nc.vector.scalar_tensor_tensor(
                            out=es[:], in0=es[:], scalar=NEG, in1=es[:],
                            op0=AOP.mult, op1=AOP.max)
                        exs = ep.tile([128, H], F32, tag="exs")
                        nc.scalar.activation(out=exs[:], in_=es[:], func=ACT.Exp)
                        den = ep.tile([128, H], F32, tag="den")
                        nc.vector.tensor_tensor(out=den[:], in0=ps[:, F:F + H],
                                                in1=exs[:], op=AOP.add)
                        rec = ep.tile([128, H], F32, tag="rec")
                        nc.vector.reciprocal(out=rec[:], in_=den[:])
                        num = ep.tile([128, F], F32, tag="num")
                        exs_b = bass.AP(tensor=exs.tensor, offset=exs[:].offset,
                                        ap=[exs[:].ap[0], [1, H], [0, C]])
                        nc.vector.tensor_tensor(
                            out=num[:].rearrange("p (h c) -> p h c", h=H),
                            in0=hog[:, 0:F].rearrange("p (h c) -> p h c", h=H),
                            in1=exs_b, op=AOP.mult)
                        nc.vector.tensor_tensor(out=num[:], in0=num[:],
                                                in1=ps[:, 0:F], op=AOP.add)
                        O = ep.tile([128, F], F32, tag="O")
                        bb = b1b if layer == 1 else b2b
                        for hh in range(H):
                            nc.vector.scalar_tensor_tensor(
                                out=O[:, C * hh:C * (hh + 1)],
                                in0=num[:, C * hh:C * (hh + 1)],
                                scalar=rec[:, hh:hh + 1],
                                in1=bb[:, C * hh:C * (hh + 1)],
                                op0=AOP.mult, op1=AOP.add)
                        if layer == 1:
                            r1f = ep.tile([128, F], F32, tag="r1f")
                            nc.scalar.activation(out=r1f[:], in_=O[:], func=ACT.Relu)
                            h2p = epp.tile([128, F2 + 2 * H], F32, tag="h2")
                            for b in range(2):
                                trp = epp.tile([128, 128], F32, tag="tr")
                                nc.tensor.transpose(out=trp[:],
                                                    in_=r1f[:, 128 * b:128 * (b + 1)],
                                                    identity=ident[:])
                                trs = ep.tile([128, 128], BF16, tag="trs")
                                nc.scalar.copy(trs[:], trp[:])
                                nc.tensor.matmul(h2p[:], lhsT=trs[:], rhs=W2extb[:, b, :],
                                                 start=(b == 0), stop=(b == 1))
                            hx2 = ep.tile([128, ROW2S], BF16, tag="hx2")
                            nc.vector.memset(hx2[:, ROW2E:ROW2S], 0)
                            nc.scalar.copy(hx2[:, 0:F2], h2p[:, 0:F2])
                            nc.scalar.copy(hx2[:, F2:F2 + H], h2p[:, F2:F2 + H])
                            nc.scalar.copy(ad2res[:, t, :],
                                           h2p[:, F2 + H:F2 + 2 * H])
                            h2tgt, h2off = hloc_rows(hext2_localA, hext2_localB, t)
                            nc.sync.dma_start(
                                out=h2tgt[h2off:h2off + 128, :],
                                in_=hx2[:])
                        else:
                            osb = ep.tile([128, F2], F32, tag="osb")
                            nc.scalar.copy(osb[:], O[:])
                            nc.sync.dma_start(out=t_out[128 * t:128 * (t + 1), :],
                                              in_=osb[:])

        agg_layer(1)
        nc.gpsimd.collective_compute(
            "AllGather", AOP.bypass, replica_groups=[list(range(cfg.NCORES))],
            ins=[hext2_localA.opt()], outs=[hext2_fullA.opt()])
        nc.gpsimd.collective_compute(
            "AllGather", AOP.bypass, replica_groups=[list(range(cfg.NCORES))],
            ins=[hext2_localB.opt()], outs=[hext2_fullB.opt()])
        agg_layer(2)

        const.release()
        dram.release()

    nc.compile()
    return nc


def make_inputs(cfg: Cfg, inputs, per_core):
    x = np.asarray(inputs["x"], np.float32)
    W1 = np.asarray(inputs["W1"], np.float32)
    as1 = np.asarray(inputs["att_src1"], np.float32)
    ad1 = np.asarray(inputs["att_dst1"], np.float32)
    b1 = np.asarray(inputs["b1"], np.float32)
    W2 = np.asarray(inputs["W2"], np.float32)
    as2 = np.asarray(inputs["att_src2"], np.float32)
    ad2 = np.asarray(inputs["att_dst2"], np.float32)
    b2 = np.asarray(inputs["b2"], np.float32)
    H, HID, CLS, F1, F2 = cfg.H, cfg.HID, cfg.CLS, cfg.F1, cfg.F2

    def ablock(ats, atd, C, F):
        A = np.zeros((F, 2 * H), np.float32)
        for hh in range(H):
            A[hh * C:(hh + 1) * C, hh] = ats[hh]
            A[hh * C:(hh + 1) * C, H + hh] = atd[hh]
        return A

    A1 = ablock(as1, ad1, HID, F1)
    A1sb = A1.reshape(2, 128, 2 * H).transpose(1, 0, 2).reshape(128, 4 * H)
    A2 = ablock(as2, ad2, CLS, F2)
    A2sb = np.zeros((128, 4 * H), np.float32)
    A2sb[:, 0:2 * H] = A2[0:128]
    A2sb[0:F2 - 128, 2 * H:4 * H] = A2[128:F2]
    W2sb = W2.reshape(2, 128, F2).transpose(1, 0, 2).reshape(128, 2 * F2)

    xpad = np.zeros((cfg.NPAD, cfg.FIN), np.float32)
    xpad[:cfg.N] = x

    in_maps = []
    for k in range(cfg.NCORES):
        xT = np.ascontiguousarray(
            xpad[k * cfg.NSH:(k + 1) * cfg.NSH].T.astype(np.float32))
        m = dict(xT=xT, W1=W1, A1=A1sb, b1=b1[None, :], W2=W2sb, A2=A2sb,
                 b2=b2[None, :], **per_core[k])
        in_maps.append(m)
    return in_maps


_CACHE = {}
LAST_RESULTS = None


def kernel(**inputs) -> np.ndarray:
    global LAST_RESULTS
    cfg = Cfg()
    edge_index = np.asarray(inputs["edge_index"])
    key = ("full",)
    if key not in _CACHE:
        groups, per_core, sizes = build_plan(cfg, edge_index)
        nc = build_program(cfg, groups, sizes)
        _CACHE[key] = (nc, groups, per_core, sizes)
    nc, groups, per_core, sizes = _CACHE[key]
    in_maps = make_inputs(cfg, inputs, per_core)
    res = bass_utils.run_bass_kernel_spmd(nc, in_maps, core_ids=list(range(cfg.NCORES)))
    LAST_RESULTS = res
    outs = [res.results[k]["out"] for k in range(cfg.NCORES)]
    full = np.concatenate(outs, axis=0)[:cfg.N]
    return full.astype(np.float32)


# revision 22
# speedup vs baseline: 1.0746x; 1.0746x over previous
# GAT (2-layer, PyG-faithful) on 8 Trainium2 NeuronCores.
#
# Strategy (graph/data parallel, per sharding hint):
#  - Nodes padded to NPAD = 8*NSH; core k owns dst nodes [k*NSH, (k+1)*NSH).
#  - Edges partitioned by dst core, chunked into 128-edge chunks per 128-dst
#    tile (lo/hi split at src=32768 for int16 gather indices).
#  - Per layer: per-shard rows [h bf16 | a_src bf16] are written to a
#    256B-strided table, AllGathered, then per-edge rows fetched with
#    dma_gather reading only the used prefix (520B/336B) of each row.
#  - Segment softmax denominators and weighted message sums accumulate in
#    PSUM via one-hot matmuls. One-hot matrices (OH: edge->dst scatter;
#    OHT: per-edge a_dst select) are precomputed on host in fp8 and
#    streamed from DRAM (graph is static).
#  - Feature standardization is folded into W1 (rows scaled by 1/std, bias
#    via K=1 matmul of -mean@W1), so x is consumed only as xT (host-
#    pretransposed) by bf16 stage-1 matmuls. Column stats come from ACT
#    passes with accum_out + a tiny AllReduce.
#  - Self-loops are handled analytically per dst tile (no gather needed).
import math
from dataclasses import dataclass, field

import numpy as np
import ml_dtypes

import concourse.bass as bass
import concourse.bacc as bacc
import concourse.tile as tile
from concourse import mybir
from concourse import bass_utils
from concourse import ap_utils
from concourse.bass import MemorySpace
from concourse.masks import make_identity

F32 = mybir.dt.float32
BF16 = mybir.dt.bfloat16
FP8 = mybir.dt.float8e4
I16 = mybir.dt.int16
AOP = mybir.AluOpType
ACT = mybir.ActivationFunctionType
NEG = 0.2
NP_FP8 = ml_dtypes.float8_e4m3


def dma_gather_raw(eng, out_ap, in_ap, idxs_ap, num_idxs, elem_size, elem_step,
                   queue_num=0):
    """dma_gather with elem_size free of the %256 restriction (which only
    applies to transpose mode); elem_step (table row stride) must be %256B."""
    assert idxs_ap.dtype == mybir.dt.int16
    assert in_ap.dtype == out_ap.dtype
    assert in_ap.space == MemorySpace.DRAM
    assert idxs_ap.space == MemorySpace.SBUF
    assert out_ap.space == MemorySpace.SBUF
    assert ap_utils.ap_is_contiguous(in_ap.ap[1:])
    assert ap_utils.ap_is_contiguous(out_ap.ap[1:])
    assert ap_utils.ap_is_contiguous(idxs_ap.ap[1:])
    assert in_ap.ap[-1][1] == elem_size == out_ap.ap[-1][1]
    assert in_ap.ap[0][0] == elem_step
    stride_bytes = elem_step * mybir.dt.size(in_ap.dtype)
    stride_bytes_256 = stride_bytes // 256
    assert stride_bytes % 256 == 0 and stride_bytes_256 < 256
    return eng.add_instruction(
        mybir.InstDMAGatherAnt(
            name=eng.bass.get_next_instruction_name(),
            ins=[*eng.lower_ap_dma(in_ap, for_custom_bir_dma=True),
                 eng.lower_ap(idxs_ap),
                 eng.lower_val_access(eng.to_reg(num_idxs))],
            outs=[eng.lower_ap(out_ap)],
            transpose=False,
            num_idxs=num_idxs,
            elem_size=elem_size,
            stride_bytes_256=stride_bytes_256,
            gen_mode=0,
            single_packet=False,
            queue_num=queue_num,
        ))


@dataclass
class Cfg:
    N: int = 50000
    FIN: int = 128
    H: int = 4
    HID: int = 64          # layer-1 per-head dim
    CLS: int = 40          # layer-2 per-head dim
    NCORES: int = 8
    SPLIT: int = 24576  # = NCORES * 24*128 (half-A rows)
    GROUP: int = 2         # dst tiles per gather-call group

    @property
    def F1(self):
        return self.H * self.HID

    @property
    def F2(self):
        return self.H * self.CLS

    @property
    def NSH(self):
        return math.ceil(self.N / (self.NCORES * 128)) * 128

    @property
    def NPAD(self):
        return self.NSH * self.NCORES

    @property
    def T(self):
        return self.NSH // 128

    @property
    def ROW1S(self):  # bf16 elems per hext1 table row (stride, %128 for 256B)
        return math.ceil((self.F1 + self.H) / 128) * 128

    @property
    def ROW1E(self):  # bf16 elems gathered per row
        return self.F1 + self.H

    @property
    def ROW2S(self):
        return math.ceil((self.F2 + self.H) / 128) * 128

    @property
    def ROW2E(self):
        return self.F2 + self.H


@dataclass
class GroupMeta:
    tiles: list
    lo_off: int          # column offset into idxlo array (int16 cols)
    nlo: int
    hi_off: int
    nhi: int
    ad_off: int          # chunk-slot offset (one-hot tables / gather chunks)
    nad: int
    tile_chunks: list = field(default_factory=list)


def build_plan(cfg: Cfg, edge_index: np.ndarray):
    """Partition edges; equalize chunk counts across cores (SPMD program is
    shared). Returns (groups_meta, per-core input dict, sizes)."""
    src = edge_index[0].astype(np.int64)
    dst = edge_index[1].astype(np.int64)
    NSH, T, NC = cfg.NSH, cfg.T, cfg.NCORES

    # half-major remap: halves (tiles [0,24) / [24,T)) of each shard are
    # grouped so each AllGather half is contiguous in the full table
    HA = 24 * 128
    ids = np.arange(cfg.NPAD, dtype=np.int64)
    kk, rr = ids // NSH, ids % NSH
    remap = np.where(rr < HA, kk * HA + rr,
                     NC * HA + kk * (NSH - HA) + (rr - HA))
    src = remap[src]

    core = dst // NSH
    tloc = (dst % NSH) // 128
    is_lo = src < cfg.SPLIT

    lists = [[[None, None] for _ in range(T)] for _ in range(NC)]
    order = np.lexsort((src, tloc, core))
    so_src, so_dst, so_core, so_tloc, so_lo = (
        src[order], dst[order], core[order], tloc[order], is_lo[order])
    for k in range(NC):
        mk = so_core == k
        for t in range(T):
            mt = mk & (so_tloc == t)
            ml = mt & so_lo
            mh = mt & ~so_lo
            lists[k][t][0] = (so_src[ml], so_dst[ml])
            lists[k][t][1] = (so_src[mh], so_dst[mh])

    Cl = [max(math.ceil(len(lists[k][t][0][0]) / 128) for k in range(NC)) for t in range(T)]
    Ch = [max(math.ceil(len(lists[k][t][1][0]) / 128) for k in range(NC)) for t in range(T)]
    Cl = [max(c, 1) for c in Cl]
    Ch = [max(c, 1) for c in Ch]

    G = cfg.GROUP
    groups = []
    lo_off = hi_off = ad_off = 0
    for g0 in range(0, T, G):
        tiles = list(range(g0, min(g0 + G, T)))
        nlo = sum(Cl[t] for t in tiles)
        nhi = sum(Ch[t] for t in tiles)
        nad = nlo + nhi
        gm = GroupMeta(tiles, lo_off, nlo, hi_off, nhi, ad_off, nad)
        slot = 0
        lo_slots = {}
        for t in tiles:
            lo_slots[t] = list(range(slot, slot + Cl[t]))
            slot += Cl[t]
        hi_slots = {}
        for t in tiles:
            hi_slots[t] = list(range(slot, slot + Ch[t]))
            slot += Ch[t]
        for t in tiles:
            gm.tile_chunks.append((t, lo_slots[t] + hi_slots[t]))
        groups.append(gm)
        lo_off += nlo * 8
        hi_off += nhi * 8
        ad_off += nad

    SLO, SHI, NCH = lo_off, hi_off, ad_off

    def wrap16(vals):
        n = len(vals)
        a = np.zeros((16, n // 16), np.int16)
        a[np.arange(n) % 16, np.arange(n) // 16] = vals
        return np.tile(a, (8, 1))

    per_core = []
    for k in range(NC):
        idxlo = np.zeros((128, SLO), np.int16)
        idxhi = np.zeros((128, SHI), np.int16)
        dl_all = np.full((128, NCH), -1.0, np.float32)
        for gm in groups:
            lo_stream = []
            hi_stream = []
            dl = np.full((128, gm.nad), -1.0, np.float32)
            slot = 0
            for cls in (0, 1):
                for t in gm.tiles:
                    s_, d_ = lists[k][t][cls]
                    nch = Cl[t] if cls == 0 else Ch[t]
                    npadded = nch * 128
                    sp = np.zeros(npadded, np.int64)
                    sp[:len(s_)] = s_ if cls == 0 else s_ - cfg.SPLIT
                    dlp = np.full(npadded, -1.0, np.float32)
                    dlp[:len(d_)] = (d_ % NSH) % 128
                    (lo_stream if cls == 0 else hi_stream).append(sp)
                    dl[:, slot:slot + nch] = dlp.reshape(nch, 128).T
                    slot += nch
            lo_v = np.concatenate(lo_stream) if lo_stream else np.zeros(0, np.int64)
            hi_v = np.concatenate(hi_stream) if hi_stream else np.zeros(0, np.int64)
            if len(lo_v):
                idxlo[:, gm.lo_off:gm.lo_off + gm.nlo * 8] = wrap16(lo_v)
            if len(hi_v):
                idxhi[:, gm.hi_off:gm.hi_off + gm.nhi * 8] = wrap16(hi_v)
            dl_all[:, gm.ad_off:gm.ad_off + gm.nad] = dl
        # one-hot tables, fp8, interleaved per slot: [oh | oht].
        #   oh[p, s, d] = (dl[p, s] == d)   (edge->dst scatter)
        #   oht[p, s, e] = (dl[e, s] == p)  (dst-select per edge)
        ar = np.arange(128, dtype=np.float32)
        ohc = np.zeros((128, NCH, 2, 128), NP_FP8)
        ohc[:, :, 0, :] = (dl_all[:, :, None] == ar[None, None, :]).astype(NP_FP8)
        ohc[:, :, 1, :] = (ar[:, None, None] == dl_all.T[None, :, :]).astype(NP_FP8)
        per_core.append(dict(idxlo=idxlo, idxhi=idxhi,
                             ohc=np.ascontiguousarray(ohc.reshape(128, NCH * 256))))
    return groups, per_core, (SLO, SHI, NCH)


def build_program(cfg: Cfg, groups, sizes):
    SLO, SHI, NCH = sizes
    H, F1, F2, HID, CLS = cfg.H, cfg.F1, cfg.F2, cfg.HID, cfg.CLS
    NSH, NPAD, T = cfg.NSH, cfg.NPAD, cfg.T
    ROW1S, ROW1E, ROW2S, ROW2E = cfg.ROW1S, cfg.ROW1E, cfg.ROW2S, cfg.ROW2E
    K1 = cfg.FIN
    assert K1 == 128
    NREAL = cfg.N

    nc = bacc.Bacc("TRN2", target_bir_lowering=False, debug=False,
                   num_devices=cfg.NCORES, num_swdge_queues=4)
    t_xT = nc.dram_tensor("xT", [128, NSH], F32, kind="ExternalInput").ap()
    t_W1 = nc.dram_tensor("W1", [K1, F1], F32, kind="ExternalInput").ap()
    t_A1 = nc.dram_tensor("A1", [128, 2 * 2 * H], F32, kind="ExternalInput").ap()
    t_b1 = nc.dram_tensor("b1", [1, F1], F32, kind="ExternalInput").ap()
    t_W2 = nc.dram_tensor("W2", [128, 2 * F2], F32, kind="ExternalInput").ap()
    t_A2 = nc.dram_tensor("A2", [128, 2 * 2 * H], F32, kind="ExternalInput").ap()
    t_b2 = nc.dram_tensor("b2", [1, F2], F32, kind="ExternalInput").ap()
    t_idxlo = nc.dram_tensor("idxlo", [128, SLO], I16, kind="ExternalInput").ap()
    t_idxhi = nc.dram_tensor("idxhi", [128, SHI], I16, kind="ExternalInput").ap()
    t_ohc = nc.dram_tensor("ohc", [128, NCH * 256], FP8, kind="ExternalInput").ap()
    t_out = nc.dram_tensor("out", [NSH, F2], F32, kind="ExternalOutput").ap()

    with tile.TileContext(nc) as tc:
        const = tc.alloc_tile_pool(name="const", bufs=1)
        dram = tc.alloc_tile_pool(name="dram", bufs=1, space="DRAM")

        HA = 24 * 128
        HB = NSH - HA
        hext1_localA = dram.tile([HA, ROW1S], BF16)
        hext1_localB = dram.tile([HB, ROW1S], BF16)
        hext1_fullA = dram.tile([cfg.NCORES * HA, ROW1S], BF16, addr_space="Shared")
        hext1_fullB = dram.tile([cfg.NCORES * HB, ROW1S], BF16, addr_space="Shared")
        hext2_localA = dram.tile([HA, ROW2S], BF16)
        hext2_localB = dram.tile([HB, ROW2S], BF16)
        hext2_fullA = dram.tile([cfg.NCORES * HA, ROW2S], BF16, addr_space="Shared")
        hext2_fullB = dram.tile([cfg.NCORES * HB, ROW2S], BF16, addr_space="Shared")

        def hloc_rows(hA, hB, t):
            if t < 24:
                return hA, 128 * t
            return hB, 128 * (t - 24)

        ident = const.tile([128, 128], F32)
        make_identity(nc, ident)
        b1b = const.tile([128, F1], F32)
        nc.gpsimd.dma_start(out=b1b[:], in_=t_b1.to_broadcast([128, F1]))
        b2b = const.tile([128, F2], F32)
        nc.gpsimd.dma_start(out=b2b[:], in_=t_b2.to_broadcast([128, F2]))
        ones1 = const.tile([1, 128], BF16)
        nc.vector.memset(ones1[:], 1.0)

        # ---- W1ext [128, F1 + 2H] = [W1 | W1 @ A1blocks] ----
        with tc.tile_pool(name="wtmp", bufs=1) as wtmp, \
             tc.tile_pool(name="wpsum", bufs=1, space="PSUM") as wpsum:
            W1sb = const.tile([128, F1], F32)
            nc.sync.dma_start(out=W1sb[:], in_=t_W1[:])
            A1sb = wtmp.tile([128, 2 * 2 * H], F32, tag="a")
            nc.sync.dma_start(out=A1sb[:], in_=t_A1[:])
            n1b = F1 // 128
            W1A_ps = wpsum.tile([128, 2 * H], F32, tag="wa")
            for b in range(n1b):
                trp = wpsum.tile([128, 128], F32, tag="tr")
                nc.tensor.transpose(out=trp[:], in_=W1sb[:, 128 * b:128 * (b + 1)],
                                    identity=ident[:])
                trs = wtmp.tile([128, 128], F32, tag="trs")
                nc.vector.tensor_copy(out=trs[:], in_=trp[:])
                nc.tensor.matmul(W1A_ps[:], lhsT=trs[:],
                                 rhs=A1sb[:, 2 * H * b:2 * H * (b + 1)],
                                 start=(b == 0), stop=(b == n1b - 1))
            W1ext = const.tile([128, F1 + 2 * H], F32)
            nc.vector.tensor_copy(out=W1ext[:, 0:F1], in_=W1sb[:])
            nc.vector.tensor_copy(out=W1ext[:, F1:F1 + 2 * H], in_=W1A_ps[:])

            # ---- W2ext [128, 2, F2 + 2H] ----
            W2sb = const.tile([128, 2, F2], F32)
            nc.sync.dma_start(out=W2sb[:], in_=t_W2.rearrange("k (b f) -> k b f", b=2))
            A2sb = wtmp.tile([128, 2 * 2 * H], F32, tag="a")
            nc.sync.dma_start(out=A2sb[:], in_=t_A2[:])
            W2ext = const.tile([128, 2, F2 + 2 * H], F32)
            fo_blocks = [(0, 128)] + ([(128, F2 - 128)] if F2 > 128 else [])
            for fb in range(2):
                W2A_ps = wpsum.tile([128, 2 * H], F32, tag="wa")
                for bi, (fo0, fow) in enumerate(fo_blocks):
                    trp = wpsum.tile([128, 128], F32, tag="tr")
                    nc.tensor.transpose(out=trp[:fow, :],
                                        in_=W2sb[:, fb, fo0:fo0 + fow],
                                        identity=ident[:])
                    trs = wtmp.tile([128, 128], F32, tag="trs")
                    nc.vector.tensor_copy(out=trs[:fow, :], in_=trp[:fow, :])
                    nc.tensor.matmul(W2A_ps[:], lhsT=trs[:fow, :],
                                     rhs=A2sb[0:fow, 2 * H * bi:2 * H * (bi + 1)],
                                     start=(bi == 0), stop=(bi == len(fo_blocks) - 1))
                nc.vector.tensor_copy(out=W2ext[:, fb, 0:F2], in_=W2sb[:, fb, :])
                nc.vector.tensor_copy(out=W2ext[:, fb, F2:F2 + 2 * H], in_=W2A_ps[:])
            W2extb = const.tile([128, 2, F2 + 2 * H], BF16)
            nc.vector.tensor_copy(out=W2extb[:], in_=W2ext[:])

        # ---- column stats from xT; fold standardization into W1ext ----
        xp = tc.alloc_tile_pool(name="xp", bufs=1)
        xT = xp.tile([128, NSH], F32)
        for q, (a, b) in enumerate([(0, NSH // 2), (NSH // 2, NSH)]):
            eng = nc.sync if q == 0 else nc.scalar
            eng.dma_start(out=xT[:, a:b], in_=t_xT[:, a:b])
        W1extb = const.tile([128, F1 + 2 * H], BF16)
        bshb = const.tile([1, F1 + 2 * H], BF16)
        with tc.tile_pool(name="stt", bufs=1) as stt, \
             tc.tile_pool(name="stp", bufs=1, space="PSUM") as stp:
            sq = xp.tile([128, NSH], F32, tag="sq")
            s12 = stt.tile([128, 2], F32, tag="s12")
            nc.scalar.activation(out=sq[:], in_=xT[:], func=ACT.Copy,
                                 accum_out=s12[:, 0:1])
            nc.scalar.activation(out=sq[:], in_=xT[:], func=ACT.Square,
                                 accum_out=s12[:, 1:2])
            stat_in = dram.tile([128, 2], F32)
            stat_out = dram.tile([128, 2], F32, addr_space="Shared")
            nc.gpsimd.dma_start(out=stat_in[:], in_=s12[:])
            nc.gpsimd.collective_compute(
                "AllReduce", AOP.add, replica_groups=[list(range(cfg.NCORES))],
                ins=[stat_in.opt()], outs=[stat_out.opt()])
            sall = stt.tile([128, 2], F32, tag="sall")
            nc.sync.dma_start(out=sall[:], in_=stat_out[:])
            mean = stt.tile([128, 1], F32, tag="mean")
            nc.scalar.mul(mean[:], sall[:, 0:1], 1.0 / NREAL)
            ex2 = stt.tile([128, 1], F32, tag="ex2")
            nc.scalar.mul(ex2[:], sall[:, 1:2], 1.0 / NREAL)
            m2 = stt.tile([128, 1], F32, tag="m2")
            nc.vector.tensor_mul(out=m2[:], in0=mean[:], in1=mean[:])
            var = stt.tile([128, 1], F32, tag="var")
            nc.vector.tensor_tensor(out=var[:], in0=ex2[:], in1=m2[:], op=AOP.subtract)
            nc.scalar.mul(var[:], var[:], NREAL / (NREAL - 1.0))
            std = stt.tile([128, 1], F32, tag="std")
            nc.scalar.activation(out=std[:], in_=var[:], func=ACT.Sqrt)
            rstd = stt.tile([128, 1], F32, tag="rstd")
            nc.vector.reciprocal(out=rstd[:], in_=std[:])
            # scale W1ext rows by 1/std (applies to both W and W@A blocks)
            nc.vector.tensor_scalar(W1extb[:], W1ext[:], rstd[:], None, AOP.mult)
            negm = stt.tile([128, 1], BF16, tag="negm")
            nc.scalar.mul(negm[:], mean[:], -1.0)
            bsh_ps = stp.tile([1, F1 + 2 * H], F32, tag="bsh")
            nc.tensor.matmul(bsh_ps[:], lhsT=negm[:], rhs=W1extb[:],
                             start=True, stop=True)
            nc.vector.tensor_copy(out=bshb[:], in_=bsh_ps[:])

        xTb = xp.tile([128, NSH], BF16, tag="xtb")
        nc.vector.tensor_copy(out=xTb[:], in_=xT[:])

        ad1res = const.tile([128, T, H], BF16)
        ad2res = const.tile([128, T, H], BF16)

        idxlo_sb = const.tile([128, SLO], I16)
        nc.sync.dma_start(out=idxlo_sb[:], in_=t_idxlo[:])
        idxhi_sb = const.tile([128, SHI], I16)
        nc.sync.dma_start(out=idxhi_sb[:], in_=t_idxhi[:])



        # ---- stage 1: hext1 rows (h1 = xT.T @ W1ext + bsh) ----
        with tc.tile_pool(name="s1sb", bufs=3) as s1sb, \
             tc.tile_pool(name="s1ps", bufs=3, space="PSUM") as s1ps:
            for g0 in range(0, T, 2):
                ts = list(range(g0, min(g0 + 2, T)))
                hx = s1sb.tile([128, len(ts), ROW1S], BF16, tag="hx")
                nc.vector.memset(hx[:, :, ROW1E:ROW1S], 0)
                for i, t in enumerate(ts):
                    h1p = s1ps.tile([128, F1 + 2 * H], F32, tag="h1")
                    nc.tensor.matmul(h1p[:], lhsT=xTb[:, 128 * t:128 * (t + 1)],
                                     rhs=W1extb[:], start=True, stop=False)
                    nc.tensor.matmul(h1p[:], lhsT=ones1[:], rhs=bshb[:],
                                     start=False, stop=True)
                    nc.scalar.copy(hx[:, i, 0:F1], h1p[:, 0:F1])
                    nc.scalar.copy(hx[:, i, F1:F1 + H], h1p[:, F1:F1 + H])
                    nc.scalar.copy(ad1res[:, t, :], h1p[:, F1 + H:F1 + 2 * H])
                htgt, roff = hloc_rows(hext1_localA, hext1_localB, g0)
                nc.sync.dma_start(
                    out=htgt[roff:roff + 128 * len(ts), :].rearrange(
                        "(a p) r -> p a r", p=128),
                    in_=hx[:])
        xp.release()

        nc.gpsimd.collective_compute(
            "AllGather", AOP.bypass, replica_groups=[list(range(cfg.NCORES))],
            ins=[hext1_localA.opt()], outs=[hext1_fullA.opt()])
        nc.gpsimd.collective_compute(
            "AllGather", AOP.bypass, replica_groups=[list(range(cfg.NCORES))],
            ins=[hext1_localB.opt()], outs=[hext1_fullB.opt()])

        qrr = [0]

        def agg_layer(layer):
            if layer == 1:
                ROWS, ROWE, F, C, hfA, hfB, hlA, hlB, adres = (
                    ROW1S, ROW1E, F1, HID, hext1_fullA, hext1_fullB,
                    hext1_localA, hext1_localB, ad1res)
            else:
                ROWS, ROWE, F, C, hfA, hfB, hlA, hlB, adres = (
                    ROW2S, ROW2E, F2, CLS, hext2_fullA, hext2_fullB,
                    hext2_localA, hext2_localB, ad2res)
            with tc.tile_pool(name=f"ag{layer}", bufs=3) as ag, \
                 tc.tile_pool(name=f"agp{layer}", bufs=2, space="PSUM") as agp, \
                 tc.tile_pool(name=f"ep{layer}", bufs=3) as ep, \
                 tc.tile_pool(name=f"epp{layer}", bufs=2, space="PSUM") as epp:
                for gm in groups:
                    nlo, nhi, nad = gm.nlo, gm.nhi, gm.nad
                    il = idxlo_sb[:, gm.lo_off:gm.lo_off + nlo * 8]
                    ih = idxhi_sb[:, gm.hi_off:gm.hi_off + nhi * 8]
                    OHC = ag.tile([128, nad, 2, 128], FP8, tag="ohc")
                    nc.sync.dma_start(
                        out=OHC[:].rearrange("p n t e -> p (n t e)"),
                        in_=t_ohc[:, gm.ad_off * 256:(gm.ad_off + nad) * 256])
                    P = ag.tile([128, nad, ROWE], BF16, tag="p")
                    in_lo = bass.AP(tensor=hfA.tensor, offset=hfA[:].offset,
                                    ap=[[ROWS, cfg.NCORES * HA], [1, ROWE]])
                    in_hi = bass.AP(tensor=hfB.tensor, offset=hfB[:].offset,
                                    ap=[[ROWS, cfg.NCORES * HB], [1, ROWE]])
                    dma_gather_raw(nc.gpsimd, P[:, 0:nlo, :], in_lo, il,
                                   nlo * 128, ROWE, ROWS, queue_num=qrr[0] % 4)
                    qrr[0] += 1
                    dma_gather_raw(nc.gpsimd, P[:, nlo:nad, :], in_hi, ih,
                                   nhi * 128, ROWE, ROWS, queue_num=qrr[0] % 4)
                    qrr[0] += 1

                    # per-edge a_dst via PE: oht.T @ adst_tile
                    slot_tile = {}
                    for (t_, slots_) in gm.tile_chunks:
                        for s_ in slots_:
                            slot_tile[s_] = t_
                    adx = agp.tile([128, nad * H], F32, tag="adx")
                    for s in range(nad):
                        nc.tensor.matmul(adx[:, H * s:H * (s + 1)],
                                         lhsT=OHC[:, s, 1, :],
                                         rhs=adres[:, slot_tile[s], :],
                                         start=True, stop=True)

                    E1 = ag.tile([128, nad, H], F32, tag="e1")
                    adxv = adx[:].rearrange("p (n h) -> p n h", h=H)
                    nc.vector.tensor_tensor(
                        out=E1[:], in0=P[:, :, F:F + H], in1=adxv, op=AOP.add)
                    nc.vector.scalar_tensor_tensor(
                        out=E1[:], in0=E1[:], scalar=NEG, in1=E1[:],
                        op0=AOP.mult, op1=AOP.max)

                    # R payload = exp(E1) broadcast over C (ACT), then *= P (DVE 2x)
                    R = ag.tile([128, nad, F + H], BF16, tag="r")
                    e1b = bass.AP(tensor=E1.tensor, offset=E1[:].offset,
                                  ap=[E1[:].ap[0], [H, nad], [1, H], [0, C]])
                    nc.scalar.activation(
                        out=R[:, :, 0:F].rearrange("p n (h c) -> p n h c", h=H),
                        in_=e1b, func=ACT.Exp)
                    nc.scalar.activation(out=R[:, :, F:F + H], in_=E1[:],
                                         func=ACT.Exp)
                    nc.vector.tensor_tensor(
                        out=R[:, :, 0:F], in0=R[:, :, 0:F],
                        in1=P[:, :, 0:F], op=AOP.mult)

                    for ti, (t, slots) in enumerate(gm.tile_chunks):
                        ps = agp.tile([128, F + H], F32, tag="acc")
                        for si, s in enumerate(slots):
                            nc.tensor.matmul(ps[:], lhsT=OHC[:, s, 0, :], rhs=R[:, s, :],
                                             start=(si == 0), stop=(si == len(slots) - 1))
                        # ---- epilogue for tile t ----
                        hog = ep.tile([128, ROWE], BF16, tag="hog")
                        hsrc, hroff = hloc_rows(hlA, hlB, t)
                        nc.sync.dma_start(out=hog[:],
                                          in_=hsrc[hroff:hroff + 128, 0:ROWE])
                        es = ep.tile([128, H], F32, tag="es")
                        nc.vector.tensor_tensor(
                            out=es[:], in0=hog[:, F:F + H],
                            in1=adres[:, t, :], op=AOP.add)
                        nc.vector.scalar_tensor_tensor(
                            out=es[:], in0=es[:], scalar=NEG, in1=es[:],
                            op0=AOP.mult, op1=AOP.max)
                        exs = ep.tile([128, H], F32, tag="exs")
                        nc.scalar.activation(out=exs[:], in_=es[:], func=ACT.Exp)
                        den = ep.tile([128, H], F32, tag="den")
                        nc.vector.tensor_tensor(out=den[:], in0=ps[:, F:F + H],
                                                in1=exs[:], op=AOP.add)
                        rec = ep.tile([128, H], F32, tag="rec")
                        nc.vector.reciprocal(out=rec[:], in_=den[:])
                        num = ep.tile([128, F], F32, tag="num")
                        exs_b = bass.AP(tensor=exs.tensor, offset=exs[:].offset,
                                        ap=[exs[:].ap[0], [1, H], [0, C]])
                        nc.vector.tensor_tensor(
                            out=num[:].rearrange("p (h c) -> p h c", h=H),
                            in0=hog[:, 0:F].rearrange("p (h c) -> p h c", h=H),
                            in1=exs_b, op=AOP.mult)
                        nc.vector.tensor_tensor(out=num[:], in0=num[:],
                                                in1=ps[:, 0:F], op=AOP.add)
                        O = ep.tile([128, F], F32, tag="O")
                        bb = b1b if layer == 1 else b2b
                        for hh in range(H):
                            nc.vector.scalar_tensor_tensor(
                                out=O[:, C * hh:C * (hh + 1)],
                                in0=num[:, C * hh:C * (hh + 1)],
                                scalar=rec[:, hh:hh + 1],
                                in1=bb[:, C * hh:C * (hh + 1)],
                                op0=AOP.mult, op1=AOP.add)
                        if layer == 1:
                            r1f = ep.tile([128, F], F32, tag="r1f")
                            nc.scalar.activation(out=r1f[:], in_=O[:], func=ACT.Relu)
                            h2p = epp.tile([128, F2 + 2 * H], F32, tag="h2")
                            for b in range(2):
                                trp = epp.tile([128, 128], F32, tag="tr")
                                nc.tensor.transpose(out=trp[:],
                                                    in_=r1f[:, 128 * b:128 * (b + 1)],
                                                    identity=ident[:])
                                trs = ep.tile([128, 128], BF16, tag="trs")
                                nc.scalar.copy(trs[:], trp[:])
                                nc.tensor.matmul(h2p[:], lhsT=trs[:], rhs=W2extb[:, b, :],
                                                 start=(b == 0), stop=(b == 1))
                            hx2 = ep.tile([128, ROW2S], BF16, tag="hx2")
                            nc.vector.memset(hx2[:, ROW2E:ROW2S], 0)
                            nc.scalar.copy(hx2[:, 0:F2], h2p[:, 0:F2])
                            nc.scalar.copy(hx2[:, F2:F2 + H], h2p[:, F2:F2 + H])
                            nc.scalar.copy(ad2res[:, t, :],
                                           h2p[:, F2 + H:F2 + 2 * H])
                            h2tgt, h2off = hloc_rows(hext2_localA, hext2_localB, t)
                            nc.sync.dma_start(
                                out=h2tgt[h2off:h2off + 128, :],
                                in_=hx2[:])
                        else:
                            osb = ep.tile([128, F2], F32, tag="osb")
                            nc.scalar.copy(osb[:], O[:])
                            nc.sync.dma_start(out=t_out[128 * t:128 * (t + 1), :],
                                              in_=osb[:])

        agg_layer(1)
        nc.gpsimd.collective_compute(
            "AllGather", AOP.bypass, replica_groups=[list(range(cfg.NCORES))],
            ins=[hext2_localA.opt()], outs=[hext2_fullA.opt()])
        nc.gpsimd.collective_compute(
            "AllGather", AOP.bypass, replica_groups=[list(range(cfg.NCORES))],
            ins=[hext2_localB.opt()], outs=[hext2_fullB.opt()])
        agg_layer(2)

        const.release()
        dram.release()

    nc.compile()
    return nc


def make_inputs(cfg: Cfg, inputs, per_core):
    x = np.asarray(inputs["x"], np.float32)
    W1 = np.asarray(inputs["W1"], np.float32)
    as1 = np.asarray(inputs["att_src1"], np.float32)
    ad1 = np.asarray(inputs["att_dst1"], np.float32)
    b1 = np.asarray(inputs["b1"], np.float32)
    W2 = np.asarray(inputs["W2"], np.float32)
    as2 = np.asarray(inputs["att_src2"], np.float32)
    ad2 = np.asarray(inputs["att_dst2"], np.float32)
    b2 = np.asarray(inputs["b2"], np.float32)
    H, HID, CLS, F1, F2 = cfg.H, cfg.HID, cfg.CLS, cfg.F1, cfg.F2

    def ablock(ats, atd, C, F):
        A = np.zeros((F, 2 * H), np.float32)
        for hh in range(H):
            A[hh * C:(hh + 1) * C, hh] = ats[hh]
            A[hh * C:(hh + 1) * C, H + hh] = atd[hh]
        return A

    A1 = ablock(as1, ad1, HID, F1)
    A1sb = A1.reshape(2, 128, 2 * H).transpose(1, 0, 2).reshape(128, 4 * H)
    A2 = ablock(as2, ad2, CLS, F2)
    A2sb = np.zeros((128, 4 * H), np.float32)
    A2sb[:, 0:2 * H] = A2[0:128]
    A2sb[0:F2 - 128, 2 * H:4 * H] = A2[128:F2]
    W2sb = W2.reshape(2, 128, F2).transpose(1, 0, 2).reshape(128, 2 * F2)

    xpad = np.zeros((cfg.NPAD, cfg.FIN), np.float32)
    xpad[:cfg.N] = x

    in_maps = []
    for k in range(cfg.NCORES):
        xT = np.ascontiguousarray(
            xpad[k * cfg.NSH:(k + 1) * cfg.NSH].T.astype(np.float32))
        m = dict(xT=xT, W1=W1, A1=A1sb, b1=b1[None, :], W2=W2sb, A2=A2sb,
                 b2=b2[None, :], **per_core[k])
        in_maps.append(m)
    return in_maps


_CACHE = {}
LAST_RESULTS = None


def kernel(**inputs) -> np.ndarray:
    global LAST_RESULTS
    cfg = Cfg()
    edge_index = np.asarray(inputs["edge_index"])
    key = ("full",)
    if key not in _CACHE:
        groups, per_core, sizes = build_plan(cfg, edge_index)
        nc = build_program(cfg, groups, sizes)
        _CACHE[key] = (nc, groups, per_core, sizes)
    nc, groups, per_core, sizes = _CACHE[key]
    in_maps = make_inputs(cfg, inputs, per_core)
    res = bass_utils.run_bass_kernel_spmd(nc, in_maps, core_ids=list(range(cfg.NCORES)))
    LAST_RESULTS = res
    outs = [res.results[k]["out"] for k in range(cfg.NCORES)]
    full = np.concatenate(outs, axis=0)[:cfg.N]
    return full.astype(np.float32)
